# revision 1
# baseline (speedup 1.0000x reference)
"""GAT (3-layer, 4-head) forward on 8 Trainium2 NeuronCores.

Strategy: partition nodes by destination across 8 cores (graph-parallel),
renumber nodes so each core's shard is degree-sorted, route edges to the
dst-owning core in a degree-slot layout (slot = (dst_local partition, round)).
Segment softmax + scatter then reduce to plain PSUM accumulation of
exp-scaled gathered rows via an identity matmul. Attention coefficients
al_src/al_dst are folded into the dense-phase weight matrix as extra output
columns, so one bulk dma_gather per block fetches everything per edge.
Halo exchange of h rows via AllGather each layer.
"""
import sys
sys.path.insert(0, "/opt/trn_rl_repo")
import numpy as np

# ---- problem constants (hardcoded per contest contract) ----
N = 50000
E = 800000
D = 128
H = 4
C = 64
HC = 256
B = 64
OUT = 10
SLOPE = 0.2

NCOR = 8
NLOC = N // NCOR          # 6250
PBLK = 128
NBLK = (NLOC + PBLK - 1) // PBLK      # 49
NLOCP = NBLK * PBLK       # 6272 padded local nodes
NTAB = NCOR * NLOCP       # 50176 global padded table rows
ROW = 320                 # legacy f32 row (unused in bf16 mode)
WCOL = 264                # Waug output cols: 256 h | 4 als | 4 ald
RB = 384                  # bf16 cols per h row (768B): 256 h | 16 al-f32-bits | pad
SPLIT = 32768             # int16 gather index limit
F32 = np.float32


# ======================================================================
# host-side preprocessing
# ======================================================================

def _wrap16(idx_flat):
    """dma_gather index layout: idx i at [p, s] with p=i%16 (replicated across
    the 8 groups of 16 partitions), s=i//16."""
    n = idx_flat.size
    assert n % 16 == 0
    a = idx_flat.reshape(n // 16, 16).T.astype(np.int16)   # [16, n/16]
    return np.tile(a, (8, 1))                              # [128, n/16]


def preprocess(x, edge_index, batch):
    """Returns per-core host data + shared static structure."""
    src0 = edge_index[0].astype(np.int64)
    dst0 = edge_index[1].astype(np.int64)
    loop = np.arange(N, dtype=np.int64)
    src = np.concatenate([src0, loop])
    dst = np.concatenate([dst0, loop])

    deg = np.bincount(dst, minlength=N)

    # new node ids: core = old // NLOC ; within core sort by degree (stable)
    newid = np.empty(N, dtype=np.int64)
    perm_per_core = []    # old local order for each core (newid -> old)
    for c in range(NCOR):
        lo, hi = c * NLOC, (c + 1) * NLOC
        order = np.argsort(deg[lo:hi], kind="stable")      # ascending degree
        perm_per_core.append(order + lo)                   # new local i -> old global
        newid[lo + order] = c * NLOCP + np.arange(NLOC)
    src_n = newid[src]
    dst_n = newid[dst]

    core_of = dst_n // NLOCP
    dloc = dst_n % NLOCP
    blk = dloc // PBLK
    p = dloc % PBLK
    islo = src_n < SPLIT

    # per (core, blk, p, half) counts
    # key for grouping; lo edges first within each (core,blk,p)
    key = ((core_of * NBLK + blk) * PBLK + p) * 2 + (1 - islo)
    order = np.argsort(key, kind="stable")
    key_s = key[order]
    src_s = src_n[order]
    # rank within group
    grp_start = np.r_[0, np.flatnonzero(np.diff(key_s)) + 1]
    grp_id = np.zeros(key_s.size, dtype=np.int64)
    grp_id[grp_start[1:]] = 1
    grp_id = np.cumsum(grp_id)
    rank = np.arange(key_s.size) - grp_start[grp_id]

    cnt = np.bincount(key, minlength=NCOR * NBLK * PBLK * 2).reshape(
        NCOR, NBLK, PBLK, 2)
    locnt = cnt[:, :, :, 0]
    hicnt = cnt[:, :, :, 1]
    TLO = locnt.max(axis=(0, 2)).astype(int)   # [NBLK] shared across cores
    THI = hicnt.max(axis=(0, 2)).astype(int)
    T = TLO + THI
    off = np.r_[0, np.cumsum(T)]               # col offset per block
    TOT = int(off[-1])

    # slot tables per core
    idx_cols = 8 * TOT  # wrapped int16 cols
    idx_all = np.zeros((NCOR, PBLK, idx_cols), dtype=np.int16)
    maskmul = np.zeros((NCOR, PBLK, TOT), dtype=F32)

    core_s = key_s // (2 * PBLK * NBLK)
    rem = key_s % (2 * PBLK * NBLK)
    blk_s = rem // (2 * PBLK)
    rem2 = rem % (2 * PBLK)
    p_s = rem2 // 2
    islo_s = (rem2 % 2) == 0

    # build flat slot-index arrays per (core, block, half)
    for c in range(NCOR):
        msk_c = core_s == c
        flat_lo = {}
        flat_hi = {}
        for b in range(NBLK):
            if TLO[b]:
                flat_lo[b] = np.zeros(TLO[b] * PBLK, dtype=np.int16)
            if THI[b]:
                flat_hi[b] = np.zeros(THI[b] * PBLK, dtype=np.int16)
        sel = np.flatnonzero(msk_c)
        bb = blk_s[sel]
        pp_ = p_s[sel]
        tt = rank[sel]
        ss = src_s[sel]
        ilo = islo_s[sel]
        # lo
        li = ilo
        for arr_sel, flat, base in ((li, flat_lo, 0), (~li, flat_hi, SPLIT)):
            idxs = np.flatnonzero(arr_sel)
            if idxs.size == 0:
                continue
            b_e = bb[idxs]
            slot = tt[idxs] * PBLK + pp_[idxs]
            val = (ss[idxs] - base).astype(np.int16)
            for b in np.unique(b_e):
                m = b_e == b
                flat[int(b)][slot[m]] = val[m]
        # mask + wrapped idx
        for b in range(NBLK):
            o = off[b]
            # lo rounds
            t_lo = np.arange(TLO[b])
            lod = locnt[c, b]                      # [128]
            maskmul[c, :, o:o + TLO[b]] = (t_lo[None, :] < lod[:, None])
            hid = hicnt[c, b]
            t_hi = np.arange(THI[b])
            maskmul[c, :, o + TLO[b]:o + T[b]] = (t_hi[None, :] < hid[:, None])
            if TLO[b]:
                idx_all[c, :, o * 8:(o + TLO[b]) * 8] = _wrap16(flat_lo[b])
            if THI[b]:
                idx_all[c, :, (o + TLO[b]) * 8:(o + T[b]) * 8] = _wrap16(flat_hi[b])

    # batch / pooling metadata in new order
    counts = np.bincount(batch.astype(np.int64), minlength=B).astype(F32)
    counts = np.maximum(counts, 1.0)
    batchcol = np.zeros((NCOR, PBLK, NBLK), dtype=F32)
    invcnt = np.zeros((NCOR, PBLK, NBLK), dtype=F32)
    xT0 = np.zeros((NCOR, D, NLOCP), dtype=F32)
    for c in range(NCOR):
        old = perm_per_core[c]                    # [NLOC] old global ids
        bt = batch[old].astype(np.int64)          # [NLOC]
        bc = np.zeros(NLOCP, dtype=F32)
        ic = np.zeros(NLOCP, dtype=F32)
        bc[:NLOC] = bt
        ic[:NLOC] = 1.0 / counts[bt]
        batchcol[c] = bc.reshape(NBLK, PBLK).T
        invcnt[c] = ic.reshape(NBLK, PBLK).T
        xT0[c, :, :NLOC] = x[old].T

    static = dict(TLO=TLO, THI=THI, T=T, off=off, TOT=TOT)
    percore = dict(idx_all=idx_all, maskmul=maskmul, batchcol=batchcol,
                   invcnt=invcnt, xT0=xT0)
    return static, percore


def make_waug(W, a_s, a_d):
    cin = W.shape[0]
    als = np.stack([W[:, h * C:(h + 1) * C] @ a_s[h] for h in range(H)], axis=1)
    ald = np.stack([W[:, h * C:(h + 1) * C] @ a_d[h] for h in range(H)], axis=1)
    return np.concatenate([W, als, ald], axis=1).astype(F32)


# ======================================================================
# bass program
# ======================================================================

def build_program(static):
    import os
    import concourse.bacc as bacc
    import concourse.bass as bass
    import concourse.mybir as mybir
    import concourse.tile as tile
    from concourse.masks import make_identity
    from concourse.library_config import mlp

    f32 = mybir.dt.float32
    AFT = mybir.ActivationFunctionType
    ALU = mybir.AluOpType
    TLO, THI, T, off, TOT = (static[k] for k in ("TLO", "THI", "T", "off", "TOT"))
    TMAX = int(T.max())

    n_layers = int(os.environ.get("GAT_LAYERS", "3"))
    n_rep = int(os.environ.get("GAT_REPEAT", "1"))
    lvl = int(os.environ.get("GAT_EDGEOPS", "9"))
    blk_cap = int(os.environ.get("GAT_BLOCKS", str(NBLK)))
    no_cc = os.environ.get("GAT_NOCC", "0") == "1"
    nc = bacc.Bacc(None, target_bir_lowering=False, num_devices=NCOR)

    # ---- I/O ----
    xT0_d = nc.dram_tensor("xT0", [D, NLOCP], f32, kind="ExternalInput")
    w_d = {}
    for ell, cin in ((0, D), (1, HC), (2, HC)):
        w_d[ell] = nc.dram_tensor(f"Waug{ell}", [cin, WCOL], f32, kind="ExternalInput")
    brep_d = {0: nc.dram_tensor("b0rep", [PBLK, HC], f32, kind="ExternalInput"),
              1: nc.dram_tensor("b1rep", [PBLK, HC], f32, kind="ExternalInput"),
              2: nc.dram_tensor("b2rep", [PBLK, C], f32, kind="ExternalInput")}
    idx_d = nc.dram_tensor("idx_all", [PBLK, 8 * TOT], mybir.dt.int16, kind="ExternalInput")
    msk_d = nc.dram_tensor("maskmul", [PBLK, TOT], f32, kind="ExternalInput")
    bcol_d = nc.dram_tensor("batchcol", [PBLK, NBLK], f32, kind="ExternalInput")
    icnt_d = nc.dram_tensor("invcnt", [PBLK, NBLK], f32, kind="ExternalInput")
    iota64_d = nc.dram_tensor("iota64", [PBLK, B], f32, kind="ExternalInput")
    pw1_d = nc.dram_tensor("pW1", [C, C // 2], f32, kind="ExternalInput")
    pb1_d = nc.dram_tensor("pb1", [C // 2, 1], f32, kind="ExternalInput")
    pw2_d = nc.dram_tensor("pW2", [C // 2, OUT], f32, kind="ExternalInput")
    pb2_d = nc.dram_tensor("pb2", [OUT, 1], f32, kind="ExternalInput")
    out_d = nc.dram_tensor("out_t", [OUT, B], f32, kind="ExternalOutput")

    # ---- internals ----
    bf16 = mybir.dt.bfloat16
    h_loc = [nc.dram_tensor(f"h_loc{l}", [NLOCP, RB], bf16) for l in range(3)]
    ag = [nc.dram_tensor(f"ag{l}", [NTAB, RB], bf16, addr_space="Shared")
          for l in range(3)]
    xT_n = [None,
            nc.dram_tensor("xT1", [HC, NLOCP], f32),
            nc.dram_tensor("xT2", [HC, NLOCP], f32)]
    pool_in = nc.dram_tensor("pool_in", [C, B], f32)
    pool_out = nc.dram_tensor("pool_out", [C, B], f32, addr_space="Shared")

    groups = [list(range(NCOR))]

    with tile.TileContext(nc) as tc:
        with tc.tile_pool(name="const", bufs=1) as cp, \
             tc.tile_pool(name="meta", bufs=1) as mp, \
             tc.tile_pool(name="gbuf", bufs=2) as gp, \
             tc.tile_pool(name="work", bufs=3) as wp, \
             tc.tile_pool(name="dense", bufs=3) as dp, \
             tc.tile_pool(name="psum_m", bufs=2, space="PSUM") as pm, \
             tc.tile_pool(name="psum_d", bufs=2, space="PSUM") as pd, \
             tc.tile_pool(name="psum_t", bufs=2, space="PSUM") as pt, \
             tc.tile_pool(name="psum_g", bufs=1, space="PSUM") as pg:

            nc.gpsimd.load_library(mlp)

            ident = cp.tile([PBLK, PBLK], dtype=f32)
            make_identity(nc, ident[:])
            identb = cp.tile([PBLK, PBLK], dtype=bf16)
            nc.vector.tensor_copy(out=identb[:], in_=ident[:])

            idx_sb = mp.tile([PBLK, 8 * TOT], dtype=mybir.dt.int16)
            nc.sync.dma_start(out=idx_sb[:], in_=idx_d[:])
            msk_sb = mp.tile([PBLK, TOT], dtype=f32)
            nc.sync.dma_start(out=msk_sb[:], in_=msk_d[:])
            bcol_sb = cp.tile([PBLK, NBLK], dtype=f32)
            nc.sync.dma_start(out=bcol_sb[:], in_=bcol_d[:])
            icnt_sb = cp.tile([PBLK, NBLK], dtype=f32)
            nc.sync.dma_start(out=icnt_sb[:], in_=icnt_d[:])
            iota_sb = cp.tile([PBLK, B], dtype=f32)
            nc.sync.dma_start(out=iota_sb[:], in_=iota64_d[:])

            pool_ps = pg.tile([C, B], dtype=f32, space="PSUM")

            for rep, ell in [(r, l) for r in range(n_rep)
                             for l in range(n_layers)]:
                cin = D if ell == 0 else HC
                nchunk = cin // PBLK
                # ---------- dense phase ----------
                wsb = []
                for ccn in range(nchunk):
                    wt = dp.tile([PBLK, WCOL], dtype=f32, tag="wsb")
                    nc.sync.dma_start(
                        out=wt[:], in_=w_d[ell][ccn * PBLK:(ccn + 1) * PBLK, :])
                    wsb.append(wt)
                xT_src = xT0_d if ell == 0 else xT_n[ell]
                for nt in range(NBLK):
                    ph = pd.tile([PBLK, WCOL], dtype=f32, space="PSUM")
                    for ccn in range(nchunk):
                        lw = dp.tile([PBLK, PBLK], dtype=f32, tag="lw")
                        nc.sync.dma_start(
                            out=lw[:],
                            in_=xT_src[ccn * PBLK:(ccn + 1) * PBLK,
                                       nt * PBLK:(nt + 1) * PBLK])
                        nc.tensor.matmul(ph[:], lhsT=lw[:], rhs=wsb[ccn][:],
                                         start=(ccn == 0), stop=(ccn == nchunk - 1))
                    hsb = dp.tile([PBLK, RB], dtype=bf16, tag="hsb")
                    nc.vector.memset(hsb[:, HC + 16:], 0.0)
                    nc.scalar.activation(hsb[:, 0:HC], ph[:, 0:HC], AFT.Copy)
                    nc.vector.tensor_copy(
                        out=hsb[:, HC:HC + 16].bitcast(f32),
                        in_=ph[:, HC:HC + 8])
                    nc.sync.dma_start(
                        out=h_loc[ell][nt * PBLK:(nt + 1) * PBLK, :], in_=hsb[:])
                # ---------- halo exchange ----------
                if no_cc:
                    nc.sync.dma_start(out=ag[ell][0:NLOCP, :], in_=h_loc[ell][:])
                else:
                    nc.gpsimd.collective_compute(
                        "AllGather", mybir.AluOpType.bypass, replica_groups=groups,
                        ins=[h_loc[ell][:]], outs=[ag[ell][:]])

                brep = cp.tile([PBLK, HC if ell < 2 else C], dtype=f32, tag=f"brep{ell}")
                nc.sync.dma_start(out=brep[:], in_=brep_d[ell][:])

                # ---------- edge phase ----------
                for b in range(blk_cap):
                    Tb, tlo, thi = int(T[b]), int(TLO[b]), int(THI[b])
                    ob = int(off[b])
                    G = gp.tile([PBLK, TMAX, RB], dtype=bf16, tag="G")
                    GCH = 8   # dma_gather caps at 1024 indices/instruction
                    for r0, r1, base in (
                        [(c0, min(c0 + GCH, tlo), 0) for c0 in range(0, tlo, GCH)]
                        + [(tlo + c0, tlo + min(c0 + GCH, thi), SPLIT)
                           for c0 in range(0, thi, GCH)]):
                        nidx = (r1 - r0) * PBLK
                        src_view = ag[ell][0:SPLIT, :] if base == 0 else ag[ell][SPLIT:, :]
                        nc.gpsimd.dma_gather(
                            G[:, r0:r1, :], src_view,
                            idx_sb[:, (ob + r0) * 8:(ob + r1) * 8],
                            nidx, nidx, RB)
                    if lvl < 2:
                        continue
                    ald8 = wp.tile([PBLK, 8], dtype=bf16, tag="ald8")
                    nc.sync.dma_start(
                        out=ald8[:],
                        in_=h_loc[ell][b * PBLK:(b + 1) * PBLK, HC + 8:HC + 16])
                    ald = ald8[:].bitcast(f32)
                    # X = exp(lrelu(als + ald)) * mask      [128, Tb, 4]
                    X = wp.tile([PBLK, TMAX, 4], dtype=f32, tag="X")
                    nc.vector.tensor_tensor(
                        out=X[:, 0:Tb, :],
                        in0=G[:, 0:Tb, HC:HC + 8].bitcast(f32)[:, :, 0:4],
                        in1=ald[:, None, :].to_broadcast([PBLK, Tb, 4]),
                        op=ALU.add)
                    Xs = wp.tile([PBLK, TMAX, 4], dtype=f32, tag="Xs")
                    nc.vector.tensor_scalar(out=Xs[:, 0:Tb, :], in0=X[:, 0:Tb, :],
                                            scalar1=SLOPE, scalar2=None,
                                            op0=ALU.mult)
                    nc.vector.tensor_tensor(out=X[:, 0:Tb, :], in0=X[:, 0:Tb, :],
                                            in1=Xs[:, 0:Tb, :], op=ALU.max)
                    nc.scalar.activation(X[:, 0:Tb, :], X[:, 0:Tb, :], AFT.Exp)
                    nc.vector.tensor_tensor(
                        out=X[:, 0:Tb, :], in0=X[:, 0:Tb, :],
                        in1=msk_sb[:, ob:ob + Tb, None].to_broadcast([PBLK, Tb, 4]),
                        op=ALU.mult)
                    if lvl < 3:
                        continue
                    Xb = wp.tile([PBLK, TMAX, 4], dtype=bf16, tag="Xb")
                    nc.vector.tensor_copy(out=Xb[:, 0:Tb, :], in_=X[:, 0:Tb, :])
                    den = wp.tile([PBLK, 4], dtype=f32, tag="den")
                    nc.vector.tensor_reduce(
                        out=den[:], in_=X[:, 0:Tb, :].rearrange("p t h -> p h t"),
                        axis=mybir.AxisListType.X, op=ALU.add)
                    # scale h cols in place per head
                    for hh in range(H):
                        nc.vector.tensor_tensor(
                            out=G[:, 0:Tb, hh * C:(hh + 1) * C],
                            in0=G[:, 0:Tb, hh * C:(hh + 1) * C],
                            in1=Xb[:, 0:Tb, hh:hh + 1].to_broadcast([PBLK, Tb, C]),
                            op=ALU.mult)
                    if lvl < 4:
                        continue
                    # accumulate [num | den] over rounds
                    M = pm.tile([PBLK, HC], dtype=f32, space="PSUM", tag="M")
                    for t in range(Tb):
                        nc.tensor.matmul(M[:], lhsT=identb[:],
                                         rhs=G[:, t, 0:HC],
                                         start=(t == 0), stop=(t == Tb - 1))
                    if lvl < 5:
                        continue
                    # normalize
                    nc.vector.tensor_scalar(out=den[:], in0=den[:],
                                            scalar1=1e-16, scalar2=None,
                                            op0=ALU.add)
                    rec = wp.tile([PBLK, 4], dtype=f32, tag="rec")
                    nc.vector.reciprocal(rec[:], den[:])
                    oh = wp.tile([PBLK, HC], dtype=f32, tag="oh")
                    nc.vector.tensor_tensor(
                        out=oh[:].rearrange("p (h c) -> p h c", h=H),
                        in0=M[:, 0:HC].rearrange("p (h c) -> p h c", h=H),
                        in1=rec[:, :, None].to_broadcast([PBLK, H, C]),
                        op=ALU.mult)
                    if lvl < 6:
                        continue
                    if ell < 2:
                        # oh = elu(oh + bias); write transposed into xT_n
                        nc.vector.tensor_tensor(out=oh[:], in0=oh[:], in1=brep[:],
                                                op=ALU.add)
                        mn = wp.tile([PBLK, HC], dtype=f32, tag="mn")
                        nc.vector.tensor_scalar(out=mn[:], in0=oh[:], scalar1=0.0,
                                                scalar2=None, op0=ALU.min)
                        ex = wp.tile([PBLK, HC], dtype=f32, tag="ex")
                        nc.scalar.activation(ex[:], mn[:], AFT.Exp)
                        nc.vector.tensor_scalar(out=ex[:], in0=ex[:], scalar1=-1.0,
                                                scalar2=None, op0=ALU.add)
                        nc.vector.tensor_tensor(out=oh[:], in0=oh[:], in1=ex[:],
                                                op=ALU.max)
                        for ccn in range(2):
                            tp = pt.tile([PBLK, PBLK], dtype=f32, space="PSUM",
                                         tag="tp")
                            nc.tensor.transpose(
                                tp[:], oh[:, ccn * PBLK:(ccn + 1) * PBLK], ident[:])
                            tps = wp.tile([PBLK, PBLK], dtype=f32, tag="tps")
                            nc.scalar.activation(tps[:], tp[:], AFT.Copy)
                            nc.sync.dma_start(
                                out=xT_n[ell + 1][ccn * PBLK:(ccn + 1) * PBLK,
                                                  b * PBLK:(b + 1) * PBLK],
                                in_=tps[:])
                    else:
                        # mean over heads + bias, then pooling contribution
                        o64 = wp.tile([PBLK, C], dtype=f32, tag="o64")
                        nc.vector.tensor_tensor(out=o64[:], in0=oh[:, 0:C],
                                                in1=oh[:, C:2 * C], op=ALU.add)
                        nc.vector.tensor_tensor(out=o64[:], in0=o64[:],
                                                in1=oh[:, 2 * C:3 * C], op=ALU.add)
                        nc.vector.tensor_tensor(out=o64[:], in0=o64[:],
                                                in1=oh[:, 3 * C:4 * C], op=ALU.add)
                        nc.vector.tensor_scalar(out=o64[:], in0=o64[:],
                                                scalar1=0.25, scalar2=None,
                                                op0=ALU.mult)
                        nc.vector.tensor_tensor(out=o64[:], in0=o64[:],
                                                in1=brep[:], op=ALU.add)
                        # scale by 1/count, build batch one-hot, accumulate
                        nc.vector.tensor_tensor(
                            out=o64[:], in0=o64[:],
                            in1=icnt_sb[:, b:b + 1].to_broadcast([PBLK, C]),
                            op=ALU.mult)
                        bh = wp.tile([PBLK, B], dtype=f32, tag="bh")
                        nc.vector.tensor_tensor(
                            out=bh[:],
                            in0=bcol_sb[:, b:b + 1].to_broadcast([PBLK, B]),
                            in1=iota_sb[:], op=ALU.is_equal)
                        nc.tensor.matmul(pool_ps[:], lhsT=o64[:], rhs=bh[:],
                                         start=(b == 0), stop=(b == blk_cap - 1))

            # ---------- pooled AllReduce + MLP ----------
            pool_sb = wp.tile([C, B], dtype=f32, tag="pool_sb")
            if n_layers == 3 and blk_cap == NBLK:
                nc.scalar.activation(pool_sb[:], pool_ps[:], AFT.Copy)
            else:
                nc.vector.memset(pool_sb[:], 0.0)
            nc.sync.dma_start(out=pool_in[:], in_=pool_sb[:])
            if no_cc:
                nc.sync.dma_start(out=pool_out[:], in_=pool_in[:])
            else:
                nc.gpsimd.collective_compute(
                    "AllReduce", mybir.AluOpType.add, replica_groups=groups,
                    ins=[pool_in[:]], outs=[pool_out[:]])
            pooled = wp.tile([C, B], dtype=f32, tag="pooled")
            nc.sync.dma_start(out=pooled[:], in_=pool_out[:])

            pw1 = cp.tile([C, C // 2], dtype=f32)
            nc.sync.dma_start(out=pw1[:], in_=pw1_d[:])
            pb1 = cp.tile([C // 2, 1], dtype=f32)
            nc.sync.dma_start(out=pb1[:], in_=pb1_d[:])
            pw2 = cp.tile([C // 2, OUT], dtype=f32)
            nc.sync.dma_start(out=pw2[:], in_=pw2_d[:])
            pb2 = cp.tile([OUT, 1], dtype=f32)
            nc.sync.dma_start(out=pb2[:], in_=pb2_d[:])

            z1p = pt.tile([C // 2, B], dtype=f32, space="PSUM", tag="tp")
            nc.tensor.matmul(z1p[:], lhsT=pw1[:], rhs=pooled[:], start=True, stop=True)
            z1 = wp.tile([C // 2, B], dtype=f32, tag="z1")
            nc.scalar.activation(z1[:], z1p[:], AFT.Relu, bias=pb1[:, 0:1])
            z2p = pt.tile([OUT, B], dtype=f32, space="PSUM", tag="tp")
            nc.tensor.matmul(z2p[:], lhsT=pw2[:], rhs=z1[:], start=True, stop=True)
            z2 = wp.tile([OUT, B], dtype=f32, tag="z2")
            nc.vector.tensor_scalar(out=z2[:], in0=z2p[:], scalar1=pb2[:, 0:1],
                                    scalar2=None, op0=ALU.add)
            nc.sync.dma_start(out=out_d[:], in_=z2[:])

    nc.compile()
    return nc


# ======================================================================
# entry point
# ======================================================================

def kernel(x, edge_index, batch, W0, b0, as0, ad0, W1, b1, as1, ad1,
           W2, b2, as2, ad2, pW1, pb1, pW2, pb2):
    x = np.asarray(x, dtype=F32)
    edge_index = np.asarray(edge_index)
    batch = np.asarray(batch)

    static, percore = preprocess(x, edge_index, batch)

    waug = {0: make_waug(np.asarray(W0, F32), np.asarray(as0, F32), np.asarray(ad0, F32)),
            1: make_waug(np.asarray(W1, F32), np.asarray(as1, F32), np.asarray(ad1, F32)),
            2: make_waug(np.asarray(W2, F32), np.asarray(as2, F32), np.asarray(ad2, F32))}
    b0r = np.broadcast_to(np.asarray(b0, F32), (PBLK, HC)).copy()
    b1r = np.broadcast_to(np.asarray(b1, F32), (PBLK, HC)).copy()
    b2r = np.broadcast_to(np.asarray(b2, F32), (PBLK, C)).copy()
    iota64 = np.broadcast_to(np.arange(B, dtype=F32), (PBLK, B)).copy()

    nc = build_program(static)

    from concourse.bass_utils import run_bass_kernel_spmd
    in_maps = []
    for c in range(NCOR):
        in_maps.append(dict(
            xT0=percore["xT0"][c],
            Waug0=waug[0], Waug1=waug[1], Waug2=waug[2],
            b0rep=b0r, b1rep=b1r, b2rep=b2r,
            idx_all=percore["idx_all"][c],
            maskmul=percore["maskmul"][c],
            batchcol=percore["batchcol"][c],
            invcnt=percore["invcnt"][c],
            iota64=iota64,
            pW1=np.asarray(pW1, F32), pb1=np.asarray(pb1, F32).reshape(-1, 1),
            pW2=np.asarray(pW2, F32), pb2=np.asarray(pb2, F32).reshape(-1, 1),
        ))
    import os as _os
    trace = _os.environ.get("GAT_TRACE", "0") == "1"
    kw = {}
    if trace:
        kw = dict(trace=True, tmpdir=_os.environ.get("GAT_TRACE_DIR") or None)
    res = run_bass_kernel_spmd(nc, in_maps, list(range(NCOR)), **kw)
    if trace:
        print(f"HW exec time: {res.exec_time_ns} ns")
    out_t = res.results[0]["out_t"]            # [OUT, B]
    return np.ascontiguousarray(out_t.T).astype(F32)



# revision 8
# speedup vs baseline: 2.2239x; 2.2239x over previous
"""GAT (3-layer, 4-head) forward on 8 Trainium2 NeuronCores.

Strategy: partition nodes by destination across 8 cores (graph-parallel),
renumber nodes so each core's shard is degree-sorted, route edges to the
dst-owning core in a degree-slot layout (slot = (dst_local partition, round)).
Segment softmax + scatter then reduce to plain PSUM accumulation of
exp-scaled gathered rows via an identity matmul. Attention coefficients
al_src/al_dst are folded into the dense-phase weight matrix as extra output
columns, so one bulk dma_gather per block fetches everything per edge.
Halo exchange of h rows via AllGather each layer.
"""
import sys
sys.path.insert(0, "/opt/trn_rl_repo")
import numpy as np

# ---- problem constants (hardcoded per contest contract) ----
N = 50000
E = 800000
D = 128
H = 4
C = 64
HC = 256
B = 64
OUT = 10
SLOPE = 0.2

NCOR = 8
NLOC = N // NCOR          # 6250
PBLK = 128
NBLK = (NLOC + PBLK - 1) // PBLK      # 49
NLOCP = NBLK * PBLK       # 6272 padded local nodes
NTAB = NCOR * NLOCP       # 50176 global padded table rows
ROW = 320                 # legacy f32 row (unused in bf16 mode)
WCOL = 264                # Waug output cols: 256 h | 4 als | 4 ald
RB = 384                  # bf16 cols per h row (768B): 256 h | 16 al-f32-bits | pad
SPLIT = 32768             # int16 gather index limit (window size)
GCH = 8                   # dma_gather caps at 1024 indices = 8 rounds
F32 = np.float32


# ======================================================================
# host-side preprocessing
# ======================================================================

def _wrap16(idx_flat):
    """dma_gather index layout: idx i at [p, s] with p=i%16 (replicated across
    the 8 groups of 16 partitions), s=i//16."""
    n = idx_flat.size
    assert n % 16 == 0
    a = idx_flat.reshape(n // 16, 16).T.astype(np.int16)   # [16, n/16]
    return np.tile(a, (8, 1))                              # [128, n/16]


def preprocess(x, edge_index, batch):
    """Returns per-core host data + shared static structure.

    Slot layout: partition p = dst-local node, round t = edge slot.
    Rounds are grouped into chunks; each chunk has a shared (across cores)
    base B so that every src index in the chunk fits the int16 window
    [B, B+32768) of the dma_gather. Per-lane edge lists are sorted by src
    and a greedy walk assigns edges to rounds, starting a new chunk when
    any lane's next edge falls outside the current window.
    """
    src0 = edge_index[0].astype(np.int64)
    dst0 = edge_index[1].astype(np.int64)
    loop = np.arange(N, dtype=np.int64)
    src = np.concatenate([src0, loop])
    dst = np.concatenate([dst0, loop])

    deg = np.bincount(dst, minlength=N)

    # new node ids: core = old // NLOC ; within core sort by degree (stable)
    newid = np.empty(N, dtype=np.int64)
    perm_per_core = []    # old local order for each core (newid -> old)
    for c in range(NCOR):
        lo, hi = c * NLOC, (c + 1) * NLOC
        order = np.argsort(deg[lo:hi], kind="stable")      # ascending degree
        perm_per_core.append(order + lo)                   # new local i -> old global
        newid[lo + order] = c * NLOCP + np.arange(NLOC)
    src_n = newid[src]
    dst_n = newid[dst]

    core_of = dst_n // NLOCP
    dloc = dst_n % NLOCP
    blk = dloc // PBLK
    p = dloc % PBLK

    # sort edges by (block, core, p, src): per-lane sorted src lists
    lane = (blk * NCOR + core_of) * PBLK + p       # block-major lane id
    order = np.lexsort((src_n, lane))
    lane_s = lane[order]
    src_s = src_n[order]
    NLANE = NCOR * PBLK
    cnt = np.bincount(lane, minlength=NBLK * NLANE)
    lane_start = np.r_[0, np.cumsum(cnt)]          # into src_s

    chunks = []        # per block: list of (r0, r1, B)
    T = np.zeros(NBLK, dtype=np.int64)
    slot_rel = []      # per block: [NLANE, T_b] int16 relative idx
    slot_msk = []      # per block: [NLANE, T_b] bool (real edge)
    for b in range(NBLK):
        base0 = b * NLANE
        starts = lane_start[base0:base0 + NLANE].copy()
        ends = lane_start[base0 + 1:base0 + NLANE + 1]
        rel_cols = []
        msk_cols = []
        bchunks = []
        Tb = 0
        while np.any(starts < ends):
            active = starts < ends
            B = int(src_s[starts[active]].min())
            B = min(B, NTAB - SPLIT)
            r0 = Tb
            for _ in range(GCH):
                active = starts < ends
                if not np.any(active):
                    break
                nxt = np.where(active, src_s[np.minimum(starts, len(src_s) - 1)],
                               np.int64(1) << 40)
                place = active & (nxt < B + SPLIT)
                rel = np.where(place, nxt - B, 0).astype(np.int16)
                rel_cols.append(rel)
                msk_cols.append(place)
                starts = starts + place
                Tb += 1
                if np.any(active & ~place):      # stalled lane -> new window
                    break
            bchunks.append((r0, Tb, B))
        chunks.append(bchunks)
        T[b] = Tb
        slot_rel.append(np.stack(rel_cols, axis=1) if Tb else
                        np.zeros((NLANE, 0), np.int16))
        slot_msk.append(np.stack(msk_cols, axis=1) if Tb else
                        np.zeros((NLANE, 0), bool))

    off = np.r_[0, np.cumsum(T)]               # col offset per block
    TOT = int(off[-1])

    # slot tables per core
    idx_cols = 8 * TOT  # wrapped int16 cols
    idx_all = np.zeros((NCOR, PBLK, idx_cols), dtype=np.int16)
    maskmul = np.zeros((NCOR, PBLK, TOT), dtype=F32)
    for b in range(NBLK):
        o = int(off[b])
        Tb = int(T[b])
        rel_b = slot_rel[b].reshape(NCOR, PBLK, Tb)
        msk_b = slot_msk[b].reshape(NCOR, PBLK, Tb)
        for c in range(NCOR):
            maskmul[c, :, o:o + Tb] = msk_b[c]
            # flat[i] for i = (t-0)*PBLK + p  ->  [Tb*PBLK]
            flat = rel_b[c].T.reshape(-1)
            idx_all[c, :, o * 8:(o + Tb) * 8] = _wrap16(flat)

    # batch / pooling metadata in new order
    counts = np.bincount(batch.astype(np.int64), minlength=B).astype(F32)
    counts = np.maximum(counts, 1.0)
    batchcol = np.zeros((NCOR, PBLK, NBLK), dtype=F32)
    invcnt = np.zeros((NCOR, PBLK, NBLK), dtype=F32)
    xT0 = np.zeros((NCOR, D, NLOCP), dtype=F32)
    for c in range(NCOR):
        old = perm_per_core[c]                    # [NLOC] old global ids
        bt = batch[old].astype(np.int64)          # [NLOC]
        bc = np.zeros(NLOCP, dtype=F32)
        ic = np.zeros(NLOCP, dtype=F32)
        bc[:NLOC] = bt
        ic[:NLOC] = 1.0 / counts[bt]
        batchcol[c] = bc.reshape(NBLK, PBLK).T
        invcnt[c] = ic.reshape(NBLK, PBLK).T
        xT0[c, :, :NLOC] = x[old].T

    static = dict(T=T, off=off, TOT=TOT, chunks=chunks)
    percore = dict(idx_all=idx_all, maskmul=maskmul, batchcol=batchcol,
                   invcnt=invcnt, xT0=xT0)
    return static, percore


def make_waug(W, a_s, a_d):
    cin = W.shape[0]
    als = np.stack([W[:, h * C:(h + 1) * C] @ a_s[h] for h in range(H)], axis=1)
    ald = np.stack([W[:, h * C:(h + 1) * C] @ a_d[h] for h in range(H)], axis=1)
    return np.concatenate([W, als, ald], axis=1).astype(F32)


# ======================================================================
# bass program
# ======================================================================

def build_program(static):
    import os
    import concourse.bacc as bacc
    import concourse.bass as bass
    import concourse.mybir as mybir
    import concourse.tile as tile
    from concourse.masks import make_identity
    from concourse.library_config import mlp

    f32 = mybir.dt.float32
    AFT = mybir.ActivationFunctionType
    ALU = mybir.AluOpType
    T, off, TOT, chunks = (static[k] for k in ("T", "off", "TOT", "chunks"))
    TMAX = int(T.max())

    n_layers = int(os.environ.get("GAT_LAYERS", "3"))
    n_rep = int(os.environ.get("GAT_REPEAT", "1"))
    lvl = int(os.environ.get("GAT_EDGEOPS", "9"))
    blk_cap = int(os.environ.get("GAT_BLOCKS", str(NBLK)))
    no_cc = os.environ.get("GAT_NOCC", "0") == "1"
    nq = int(os.environ.get("GAT_NQ", "4"))
    nc = bacc.Bacc(None, target_bir_lowering=False, num_devices=NCOR,
                   num_swdge_queues=nq)

    # ---- I/O ----
    xT0_d = nc.dram_tensor("xT0", [D, NLOCP], f32, kind="ExternalInput")
    w_d = {}
    for ell, cin in ((0, D), (1, HC), (2, HC)):
        w_d[ell] = nc.dram_tensor(f"Waug{ell}", [cin, WCOL], f32, kind="ExternalInput")
    brep_d = {0: nc.dram_tensor("b0rep", [PBLK, HC], f32, kind="ExternalInput"),
              1: nc.dram_tensor("b1rep", [PBLK, HC], f32, kind="ExternalInput"),
              2: nc.dram_tensor("b2rep", [PBLK, C], f32, kind="ExternalInput")}
    idx_d = nc.dram_tensor("idx_all", [PBLK, 8 * TOT], mybir.dt.int16, kind="ExternalInput")
    msk_d = nc.dram_tensor("maskmul", [PBLK, TOT], f32, kind="ExternalInput")
    bcol_d = nc.dram_tensor("batchcol", [PBLK, NBLK], f32, kind="ExternalInput")
    icnt_d = nc.dram_tensor("invcnt", [PBLK, NBLK], f32, kind="ExternalInput")
    iota64_d = nc.dram_tensor("iota64", [PBLK, B], f32, kind="ExternalInput")
    pw1_d = nc.dram_tensor("pW1", [C, C // 2], f32, kind="ExternalInput")
    pb1_d = nc.dram_tensor("pb1", [C // 2, 1], f32, kind="ExternalInput")
    pw2_d = nc.dram_tensor("pW2", [C // 2, OUT], f32, kind="ExternalInput")
    pb2_d = nc.dram_tensor("pb2", [OUT, 1], f32, kind="ExternalInput")
    out_d = nc.dram_tensor("out_t", [OUT, B], f32, kind="ExternalOutput")

    # ---- internals ----
    bf16 = mybir.dt.bfloat16
    h_loc = [nc.dram_tensor(f"h_loc{l}", [NLOCP, RB], bf16) for l in range(3)]
    ag = [nc.dram_tensor(f"ag{l}", [NTAB, RB], bf16, addr_space="Shared")
          for l in range(3)]
    xT_n = [None,
            nc.dram_tensor("xT1", [HC, NLOCP], f32),
            nc.dram_tensor("xT2", [HC, NLOCP], f32)]
    pool_in = nc.dram_tensor("pool_in", [C, B], f32)
    pool_out = nc.dram_tensor("pool_out", [C, B], f32, addr_space="Shared")

    groups = [list(range(NCOR))]

    with tile.TileContext(nc) as tc:
        with tc.tile_pool(name="const", bufs=1) as cp, \
             tc.tile_pool(name="meta", bufs=1) as mp, \
             tc.tile_pool(name="gbuf", bufs=2) as gp, \
             tc.tile_pool(name="work", bufs=3) as wp, \
             tc.tile_pool(name="dense", bufs=3) as dp, \
             tc.tile_pool(name="psum_m", bufs=2, space="PSUM") as pm, \
             tc.tile_pool(name="psum_d", bufs=2, space="PSUM") as pd, \
             tc.tile_pool(name="psum_t", bufs=2, space="PSUM") as pt, \
             tc.tile_pool(name="psum_g", bufs=1, space="PSUM") as pg:

            nc.gpsimd.load_library(mlp)

            ident = cp.tile([PBLK, PBLK], dtype=f32)
            make_identity(nc, ident[:])
            identb = cp.tile([PBLK, PBLK], dtype=bf16)
            nc.vector.tensor_copy(out=identb[:], in_=ident[:])

            idx_sb = mp.tile([PBLK, 8 * TOT], dtype=mybir.dt.int16)
            nc.sync.dma_start(out=idx_sb[:], in_=idx_d[:])
            msk_sb = mp.tile([PBLK, TOT], dtype=f32)
            nc.sync.dma_start(out=msk_sb[:], in_=msk_d[:])
            bcol_sb = cp.tile([PBLK, NBLK], dtype=f32)
            nc.sync.dma_start(out=bcol_sb[:], in_=bcol_d[:])
            icnt_sb = cp.tile([PBLK, NBLK], dtype=f32)
            nc.sync.dma_start(out=icnt_sb[:], in_=icnt_d[:])
            iota_sb = cp.tile([PBLK, B], dtype=f32)
            nc.sync.dma_start(out=iota_sb[:], in_=iota64_d[:])

            pool_ps = pg.tile([C, B], dtype=f32, space="PSUM")

            for rep, ell in [(r, l) for r in range(n_rep)
                             for l in range(n_layers)]:
                cin = D if ell == 0 else HC
                nchunk = cin // PBLK
                # ---------- dense phase ----------
                wsb = []
                for ccn in range(nchunk):
                    wt = dp.tile([PBLK, WCOL], dtype=f32, tag="wsb")
                    nc.sync.dma_start(
                        out=wt[:], in_=w_d[ell][ccn * PBLK:(ccn + 1) * PBLK, :])
                    wsb.append(wt)
                xT_src = xT0_d if ell == 0 else xT_n[ell]
                for nt in range(NBLK):
                    ph = pd.tile([PBLK, WCOL], dtype=f32, space="PSUM")
                    for ccn in range(nchunk):
                        lw = dp.tile([PBLK, PBLK], dtype=f32, tag="lw")
                        nc.sync.dma_start(
                            out=lw[:],
                            in_=xT_src[ccn * PBLK:(ccn + 1) * PBLK,
                                       nt * PBLK:(nt + 1) * PBLK])
                        nc.tensor.matmul(ph[:], lhsT=lw[:], rhs=wsb[ccn][:],
                                         start=(ccn == 0), stop=(ccn == nchunk - 1))
                    hsb = dp.tile([PBLK, RB], dtype=bf16, tag="hsb")
                    nc.vector.memset(hsb[:, HC + 16:], 0.0)
                    nc.scalar.activation(hsb[:, 0:HC], ph[:, 0:HC], AFT.Copy)
                    nc.vector.tensor_copy(
                        out=hsb[:, HC:HC + 16].bitcast(f32),
                        in_=ph[:, HC:HC + 8])
                    nc.sync.dma_start(
                        out=h_loc[ell][nt * PBLK:(nt + 1) * PBLK, :], in_=hsb[:])
                # ---------- halo exchange ----------
                if no_cc:
                    nc.sync.dma_start(out=ag[ell][0:NLOCP, :], in_=h_loc[ell][:])
                else:
                    nc.gpsimd.collective_compute(
                        "AllGather", mybir.AluOpType.bypass, replica_groups=groups,
                        ins=[h_loc[ell][:]], outs=[ag[ell][:]])

                brep = cp.tile([PBLK, HC if ell < 2 else C], dtype=f32, tag=f"brep{ell}")
                nc.sync.dma_start(out=brep[:], in_=brep_d[ell][:])

                # ---------- edge phase ----------
                gq = 0
                for b in range(blk_cap):
                    Tb = int(T[b])
                    ob = int(off[b])
                    G = gp.tile([PBLK, TMAX, RB], dtype=bf16, tag="G")
                    for r0, r1, base in chunks[b]:
                        nidx = (r1 - r0) * PBLK
                        src_view = ag[ell][base:base + SPLIT, :]
                        nc.gpsimd.dma_gather(
                            G[:, r0:r1, :], src_view,
                            idx_sb[:, (ob + r0) * 8:(ob + r1) * 8],
                            nidx, nidx, RB, queue_num=gq % nq)
                        gq += 1
                    if lvl < 2:
                        continue
                    ald8 = wp.tile([PBLK, 8], dtype=bf16, tag="ald8")
                    nc.sync.dma_start(
                        out=ald8[:],
                        in_=h_loc[ell][b * PBLK:(b + 1) * PBLK, HC + 8:HC + 16])
                    ald = ald8[:].bitcast(f32)
                    # X = exp(lrelu(als + ald)) * mask      [128, Tb, 4]
                    X = wp.tile([PBLK, TMAX, 4], dtype=f32, tag="X")
                    nc.vector.tensor_tensor(
                        out=X[:, 0:Tb, :],
                        in0=G[:, 0:Tb, HC:HC + 8].bitcast(f32)[:, :, 0:4],
                        in1=ald[:, None, :].to_broadcast([PBLK, Tb, 4]),
                        op=ALU.add)
                    Xs = wp.tile([PBLK, TMAX, 4], dtype=f32, tag="Xs")
                    nc.vector.tensor_scalar(out=Xs[:, 0:Tb, :], in0=X[:, 0:Tb, :],
                                            scalar1=SLOPE, scalar2=None,
                                            op0=ALU.mult)
                    nc.vector.tensor_tensor(out=X[:, 0:Tb, :], in0=X[:, 0:Tb, :],
                                            in1=Xs[:, 0:Tb, :], op=ALU.max)
                    nc.scalar.activation(X[:, 0:Tb, :], X[:, 0:Tb, :], AFT.Exp)
                    nc.vector.tensor_tensor(
                        out=X[:, 0:Tb, :], in0=X[:, 0:Tb, :],
                        in1=msk_sb[:, ob:ob + Tb, None].to_broadcast([PBLK, Tb, 4]),
                        op=ALU.mult)
                    if lvl < 3:
                        continue
                    Xb = wp.tile([PBLK, TMAX, 4], dtype=bf16, tag="Xb")
                    nc.vector.tensor_copy(out=Xb[:, 0:Tb, :], in_=X[:, 0:Tb, :])
                    den = wp.tile([PBLK, 4], dtype=f32, tag="den")
                    nc.vector.tensor_reduce(
                        out=den[:], in_=X[:, 0:Tb, :].rearrange("p t h -> p h t"),
                        axis=mybir.AxisListType.X, op=ALU.add)
                    # scale h cols in place per head
                    for hh in range(H):
                        nc.vector.tensor_tensor(
                            out=G[:, 0:Tb, hh * C:(hh + 1) * C],
                            in0=G[:, 0:Tb, hh * C:(hh + 1) * C],
                            in1=Xb[:, 0:Tb, hh:hh + 1].to_broadcast([PBLK, Tb, C]),
                            op=ALU.mult)
                    if lvl < 4:
                        continue
                    # accumulate [num | den] over rounds
                    M = pm.tile([PBLK, HC], dtype=f32, space="PSUM", tag="M")
                    for t in range(Tb):
                        nc.tensor.matmul(M[:], lhsT=identb[:],
                                         rhs=G[:, t, 0:HC],
                                         start=(t == 0), stop=(t == Tb - 1))
                    if lvl < 5:
                        continue
                    # normalize
                    nc.vector.tensor_scalar(out=den[:], in0=den[:],
                                            scalar1=1e-16, scalar2=None,
                                            op0=ALU.add)
                    rec = wp.tile([PBLK, 4], dtype=f32, tag="rec")
                    nc.vector.reciprocal(rec[:], den[:])
                    oh = wp.tile([PBLK, HC], dtype=f32, tag="oh")
                    nc.vector.tensor_tensor(
                        out=oh[:].rearrange("p (h c) -> p h c", h=H),
                        in0=M[:, 0:HC].rearrange("p (h c) -> p h c", h=H),
                        in1=rec[:, :, None].to_broadcast([PBLK, H, C]),
                        op=ALU.mult)
                    if lvl < 6:
                        continue
                    if ell < 2:
                        # oh = elu(oh + bias); write transposed into xT_n
                        nc.vector.tensor_tensor(out=oh[:], in0=oh[:], in1=brep[:],
                                                op=ALU.add)
                        mn = wp.tile([PBLK, HC], dtype=f32, tag="mn")
                        nc.vector.tensor_scalar(out=mn[:], in0=oh[:], scalar1=0.0,
                                                scalar2=None, op0=ALU.min)
                        ex = wp.tile([PBLK, HC], dtype=f32, tag="ex")
                        nc.scalar.activation(ex[:], mn[:], AFT.Exp)
                        nc.vector.tensor_scalar(out=ex[:], in0=ex[:], scalar1=-1.0,
                                                scalar2=None, op0=ALU.add)
                        nc.vector.tensor_tensor(out=oh[:], in0=oh[:], in1=ex[:],
                                                op=ALU.max)
                        for ccn in range(2):
                            tp = pt.tile([PBLK, PBLK], dtype=f32, space="PSUM",
                                         tag="tp")
                            nc.tensor.transpose(
                                tp[:], oh[:, ccn * PBLK:(ccn + 1) * PBLK], ident[:])
                            tps = wp.tile([PBLK, PBLK], dtype=f32, tag="tps")
                            nc.scalar.activation(tps[:], tp[:], AFT.Copy)
                            nc.sync.dma_start(
                                out=xT_n[ell + 1][ccn * PBLK:(ccn + 1) * PBLK,
                                                  b * PBLK:(b + 1) * PBLK],
                                in_=tps[:])
                    else:
                        # mean over heads + bias, then pooling contribution
                        o64 = wp.tile([PBLK, C], dtype=f32, tag="o64")
                        nc.vector.tensor_tensor(out=o64[:], in0=oh[:, 0:C],
                                                in1=oh[:, C:2 * C], op=ALU.add)
                        nc.vector.tensor_tensor(out=o64[:], in0=o64[:],
                                                in1=oh[:, 2 * C:3 * C], op=ALU.add)
                        nc.vector.tensor_tensor(out=o64[:], in0=o64[:],
                                                in1=oh[:, 3 * C:4 * C], op=ALU.add)
                        nc.vector.tensor_scalar(out=o64[:], in0=o64[:],
                                                scalar1=0.25, scalar2=None,
                                                op0=ALU.mult)
                        nc.vector.tensor_tensor(out=o64[:], in0=o64[:],
                                                in1=brep[:], op=ALU.add)
                        # scale by 1/count, build batch one-hot, accumulate
                        nc.vector.tensor_tensor(
                            out=o64[:], in0=o64[:],
                            in1=icnt_sb[:, b:b + 1].to_broadcast([PBLK, C]),
                            op=ALU.mult)
                        bh = wp.tile([PBLK, B], dtype=f32, tag="bh")
                        nc.vector.tensor_tensor(
                            out=bh[:],
                            in0=bcol_sb[:, b:b + 1].to_broadcast([PBLK, B]),
                            in1=iota_sb[:], op=ALU.is_equal)
                        nc.tensor.matmul(pool_ps[:], lhsT=o64[:], rhs=bh[:],
                                         start=(b == 0), stop=(b == blk_cap - 1))

            # ---------- pooled AllReduce + MLP ----------
            pool_sb = wp.tile([C, B], dtype=f32, tag="pool_sb")
            if n_layers == 3 and blk_cap == NBLK:
                nc.scalar.activation(pool_sb[:], pool_ps[:], AFT.Copy)
            else:
                nc.vector.memset(pool_sb[:], 0.0)
            nc.sync.dma_start(out=pool_in[:], in_=pool_sb[:])
            if no_cc:
                nc.sync.dma_start(out=pool_out[:], in_=pool_in[:])
            else:
                nc.gpsimd.collective_compute(
                    "AllReduce", mybir.AluOpType.add, replica_groups=groups,
                    ins=[pool_in[:]], outs=[pool_out[:]])
            pooled = wp.tile([C, B], dtype=f32, tag="pooled")
            nc.sync.dma_start(out=pooled[:], in_=pool_out[:])

            pw1 = cp.tile([C, C // 2], dtype=f32)
            nc.sync.dma_start(out=pw1[:], in_=pw1_d[:])
            pb1 = cp.tile([C // 2, 1], dtype=f32)
            nc.sync.dma_start(out=pb1[:], in_=pb1_d[:])
            pw2 = cp.tile([C // 2, OUT], dtype=f32)
            nc.sync.dma_start(out=pw2[:], in_=pw2_d[:])
            pb2 = cp.tile([OUT, 1], dtype=f32)
            nc.sync.dma_start(out=pb2[:], in_=pb2_d[:])

            z1p = pt.tile([C // 2, B], dtype=f32, space="PSUM", tag="tp")
            nc.tensor.matmul(z1p[:], lhsT=pw1[:], rhs=pooled[:], start=True, stop=True)
            z1 = wp.tile([C // 2, B], dtype=f32, tag="z1")
            nc.scalar.activation(z1[:], z1p[:], AFT.Relu, bias=pb1[:, 0:1])
            z2p = pt.tile([OUT, B], dtype=f32, space="PSUM", tag="tp")
            nc.tensor.matmul(z2p[:], lhsT=pw2[:], rhs=z1[:], start=True, stop=True)
            z2 = wp.tile([OUT, B], dtype=f32, tag="z2")
            nc.vector.tensor_scalar(out=z2[:], in0=z2p[:], scalar1=pb2[:, 0:1],
                                    scalar2=None, op0=ALU.add)
            nc.sync.dma_start(out=out_d[:], in_=z2[:])

    nc.compile()
    return nc


# ======================================================================
# entry point
# ======================================================================

def kernel(x, edge_index, batch, W0, b0, as0, ad0, W1, b1, as1, ad1,
           W2, b2, as2, ad2, pW1, pb1, pW2, pb2):
    x = np.asarray(x, dtype=F32)
    edge_index = np.asarray(edge_index)
    batch = np.asarray(batch)

    static, percore = preprocess(x, edge_index, batch)

    waug = {0: make_waug(np.asarray(W0, F32), np.asarray(as0, F32), np.asarray(ad0, F32)),
            1: make_waug(np.asarray(W1, F32), np.asarray(as1, F32), np.asarray(ad1, F32)),
            2: make_waug(np.asarray(W2, F32), np.asarray(as2, F32), np.asarray(ad2, F32))}
    b0r = np.broadcast_to(np.asarray(b0, F32), (PBLK, HC)).copy()
    b1r = np.broadcast_to(np.asarray(b1, F32), (PBLK, HC)).copy()
    b2r = np.broadcast_to(np.asarray(b2, F32), (PBLK, C)).copy()
    iota64 = np.broadcast_to(np.arange(B, dtype=F32), (PBLK, B)).copy()

    nc = build_program(static)

    from concourse.bass_utils import run_bass_kernel_spmd
    in_maps = []
    for c in range(NCOR):
        in_maps.append(dict(
            xT0=percore["xT0"][c],
            Waug0=waug[0], Waug1=waug[1], Waug2=waug[2],
            b0rep=b0r, b1rep=b1r, b2rep=b2r,
            idx_all=percore["idx_all"][c],
            maskmul=percore["maskmul"][c],
            batchcol=percore["batchcol"][c],
            invcnt=percore["invcnt"][c],
            iota64=iota64,
            pW1=np.asarray(pW1, F32), pb1=np.asarray(pb1, F32).reshape(-1, 1),
            pW2=np.asarray(pW2, F32), pb2=np.asarray(pb2, F32).reshape(-1, 1),
        ))
    import os as _os
    trace = _os.environ.get("GAT_TRACE", "0") == "1"
    kw = {}
    if trace:
        kw = dict(trace=True, tmpdir=_os.environ.get("GAT_TRACE_DIR") or None)
    res = run_bass_kernel_spmd(nc, in_maps, list(range(NCOR)), **kw)
    if trace:
        print(f"HW exec time: {res.exec_time_ns} ns")
    out_t = res.results[0]["out_t"]            # [OUT, B]
    return np.ascontiguousarray(out_t.T).astype(F32)

